# revision 12
# baseline (speedup 1.0000x reference)
"""Trainium2 Bass kernel for a 2-layer LSTM (batch 8192, seq 128, in 32, hidden 64)
with a final linear head producing one logit per batch element.

Strategy: pure data parallel over 8 NeuronCores (1024 batch each), weights
replicated.  The input projection is folded into the recurrent step (no
[B,T,4H] materialization) so HBM traffic is ~one read of x.

Single wide chain per core: the 1024 batch splits into 2 partition-halves of
512 columns.  Per gate the pre-activations live in PSUM as [128, 512] =
[gate(half A); gate(half B)], produced by block-diagonal matmuls
(stationary diag(W_G, W_G)).  The ACT (scalar) engine is the bottleneck
(sigmoid/tanh streams 1 col/cycle), so activation instructions are made as
wide as possible: per timestep only sigma(i,g) [1024 cols], sigma(f,o)
[1024], sigma(L1 all gates) [2048], tanh(c0) [512], tanh(c1) [512].
The L0 sigmoid is split (i,g | f,o) to shorten the recurrence-critical path;
L1's forget-gate bias (+1) is accumulated in PSUM by a ones-rank-1 matmul so
the L1 sigmoid is a single unbiased instruction.  Matmuls use float32r
(full-rate fp32).  tanh(g) = 2*sigmoid(2g)-1 is folded into the DVE ops with
gate weights pre-scaled by 2.
"""

import numpy as np

INPUT = 32
HIDDEN = 64
BATCH = 8192
SEQ = 128
NCORES = 8
BCORE = BATCH // NCORES      # 1024
BHC = BCORE // 2             # 512 columns (x2 partition halves)
D1 = INPUT + 1               # x rows + ones row

_CACHE = {}


def _build_module():
    import concourse.bacc as bacc
    import concourse.mybir as mybir
    import concourse.tile as tile

    F32 = mybir.dt.float32
    F32R = mybir.dt.float32r
    AF = mybir.ActivationFunctionType
    MUL = mybir.AluOpType.mult
    ADD = mybir.AluOpType.add
    SUB = mybir.AluOpType.subtract

    nc = bacc.Bacc()
    # L0 gate order in P0_ig: [i, g]; in P0_fo: [f, o].
    # L1 gate order in P1: [i, g, o, f] (f last; its +1 bias rides a rank-1
    # ones matmul).
    xT = nc.dram_tensor("xT", [SEQ, 2 * D1, BHC], F32R, kind="ExternalInput")
    wx0 = nc.dram_tensor("wx0", [4, 2 * D1, 128], F32R, kind="ExternalInput")
    wh0 = nc.dram_tensor("wh0", [4, 128, 128], F32R, kind="ExternalInput")
    w1a = nc.dram_tensor("w1a", [4, 128, 128], F32R, kind="ExternalInput")
    w1b = nc.dram_tensor("w1b", [4, 128, 128], F32R, kind="ExternalInput")
    b1st = nc.dram_tensor("b1st", [4, 1, 128], F32R, kind="ExternalInput")
    ones = nc.dram_tensor("ones", [1, BHC], F32R, kind="ExternalInput")
    fcw = nc.dram_tensor("fcw", [128, 2], F32R, kind="ExternalInput")
    fcb = nc.dram_tensor("fcb", [2, 1], F32, kind="ExternalInput")
    out = nc.dram_tensor("out", [2, BHC], F32, kind="ExternalOutput")

    with tile.TileContext(nc) as tc:
        with (
            tc.tile_pool(name="wp", bufs=1) as wp,
            tc.tile_pool(name="sb", bufs=3) as sb,
            tc.tile_pool(name="ps", bufs=1, space="PSUM") as ps,
        ):
            twx = [wp.tile([2 * D1, 128], F32R, name=f"twx{g}", tag=f"twx{g}") for g in range(4)]
            twh = [wp.tile([128, 128], F32R, name=f"twh{g}", tag=f"twh{g}") for g in range(4)]
            t1a = [wp.tile([128, 128], F32R, name=f"t1a{g}", tag=f"t1a{g}") for g in range(4)]
            t1b = [wp.tile([128, 128], F32R, name=f"t1b{g}", tag=f"t1b{g}") for g in range(4)]
            tone = wp.tile([1, BHC], F32R, name="tone")      # ones row (L1 bias)
            tb1 = [wp.tile([1, 128], F32R, name=f"tb1{g}", tag=f"tb1{g}") for g in range(4)]
            tfcw = wp.tile([128, 2], F32R, name="tfcw")
            tfcb = wp.tile([2, 1], F32, name="tfcb")
            for g in range(4):
                nc.sync.dma_start(twx[g][:, :], wx0[g, :, :])
                nc.sync.dma_start(twh[g][:, :], wh0[g, :, :])
                nc.sync.dma_start(t1a[g][:, :], w1a[g, :, :])
                nc.sync.dma_start(t1b[g][:, :], w1b[g, :, :])
                nc.sync.dma_start(tb1[g][:, :], b1st[g, :, :])
            nc.sync.dma_start(tfcw[:, :], fcw[:, :])
            nc.sync.dma_start(tfcb[:, :], fcb[:, :])
            nc.sync.dma_start(tone[:, :], ones[0:1, :])

            h1p = h2p = c0p = c1p = None
            h2_last = None

            for t in range(SEQ):
                first = t == 0

                xt = sb.tile([2 * D1, BHC], F32R, name=f"xt{t}", tag="xt", bufs=4)
                nc.sync.dma_start(xt[:, :], xT[t, :, :])

                # ---- layer 0 pre-activations: P0_ig = [i, g], P0_fo = [f, o]
                P0i = ps.tile([128, 2 * BHC], F32, name=f"P0i_{t}", tag="P0i", bufs=1)
                P0f = ps.tile([128, 2 * BHC], F32, name=f"P0f_{t}", tag="P0f", bufs=1)
                for k, P in ((0, P0i), (1, P0i), (2, P0f), (3, P0f)):
                    blk = slice(0, BHC) if k in (0, 2) else slice(BHC, 2 * BHC)
                    nc.tensor.matmul(P[:, blk], twx[k][:, :], xt[:, :],
                                     start=True, stop=first)
                    if not first:
                        nc.tensor.matmul(P[:, blk], twh[k][:, :], h1p[:, :],
                                         start=False, stop=True)

                # sigma over [i, g] then [f, o] (g pre-scaled x2 in weights;
                # tanh(g) = 2*sigma(2g)-1 folded into the DVE ops below)
                sig = sb.tile([128, 2 * BHC], F32, name=f"sig_{t}", tag="sig", bufs=2)
                nc.scalar.activation(sig[:, :], P0i[:, :], AF.Sigmoid)
                sfo = sb.tile([128, 2 * BHC], F32, name=f"sfo_{t}", tag="sfo", bufs=2)
                nc.scalar.activation(sfo[:, :], P0f[:, :], AF.Sigmoid)

                # ig_half = (sigma(2g) - 0.5) * sigma(i) = i*tanh(g)/2
                ig0 = sb.tile([128, BHC], F32, name=f"ig0_{t}", tag="ig0", bufs=2)
                nc.vector.scalar_tensor_tensor(
                    ig0[:, :], sig[:, BHC:2 * BHC], 0.5, sig[:, 0:BHC], SUB, MUL)
                if first:
                    c0 = sb.tile([128, BHC], F32, name=f"c0_{t}", tag="c0", bufs=2)
                    nc.vector.tensor_scalar_mul(c0[:, :], ig0[:, :], 2.0)
                else:
                    fc0 = sb.tile([128, BHC], F32, name=f"fc0_{t}", tag="fc0", bufs=2)
                    nc.vector.tensor_tensor(fc0[:, :], sfo[:, 0:BHC], c0p[:, :], MUL)
                    c0 = sb.tile([128, BHC], F32, name=f"c0_{t}", tag="c0", bufs=2)
                    nc.vector.scalar_tensor_tensor(
                        c0[:, :], ig0[:, :], 2.0, fc0[:, :], MUL, ADD)
                th0 = sb.tile([128, BHC], F32, name=f"th0_{t}", tag="th0", bufs=2)
                nc.scalar.activation(th0[:, :], c0[:, :], AF.Tanh)
                h1 = sb.tile([128, BHC], F32R, name=f"h1_{t}", tag="h1", bufs=2)
                nc.vector.tensor_tensor(h1[:, :], sfo[:, BHC:2 * BHC], th0[:, :], MUL)

                # ---- layer 1: P1 = [i, g, o, f] in one 4-bank PSUM tile ----
                P1 = ps.tile([128, 4 * BHC], F32, name=f"P1_{t}", tag="P1", bufs=1)
                for g in range(4):
                    blk = slice(g * BHC, (g + 1) * BHC)
                    nc.tensor.matmul(P1[:, blk], t1a[g][:, :], h1[:, :],
                                     start=True, stop=False)
                    if not first:
                        nc.tensor.matmul(P1[:, blk], t1b[g][:, :], h2p[:, :],
                                         start=False, stop=False)
                    # per-gate bias as a rank-1 ones matmul accumulation
                    nc.tensor.matmul(P1[:, blk], tb1[g][:, :], tone[:, :],
                                     start=False, stop=True)

                ga = sb.tile([128, 4 * BHC], F32, name=f"ga_{t}", tag="ga", bufs=2)
                nc.scalar.activation(ga[:, :], P1[:, :], AF.Sigmoid)

                ig1 = sb.tile([128, BHC], F32, name=f"ig1_{t}", tag="ig1", bufs=2)
                nc.vector.scalar_tensor_tensor(
                    ig1[:, :], ga[:, BHC:2 * BHC], 0.5, ga[:, 0:BHC], SUB, MUL)
                if first:
                    c1 = sb.tile([128, BHC], F32, name=f"c1_{t}", tag="c1", bufs=2)
                    nc.vector.tensor_scalar_mul(c1[:, :], ig1[:, :], 2.0)
                else:
                    fc1 = sb.tile([128, BHC], F32, name=f"fc1_{t}", tag="fc1", bufs=2)
                    nc.vector.tensor_tensor(fc1[:, :], ga[:, 3 * BHC:4 * BHC], c1p[:, :], MUL)
                    c1 = sb.tile([128, BHC], F32, name=f"c1_{t}", tag="c1", bufs=2)
                    nc.vector.scalar_tensor_tensor(
                        c1[:, :], ig1[:, :], 2.0, fc1[:, :], MUL, ADD)
                th1 = sb.tile([128, BHC], F32, name=f"th1_{t}", tag="th1", bufs=2)
                nc.scalar.activation(th1[:, :], c1[:, :], AF.Tanh)
                h2 = sb.tile([128, BHC], F32R, name=f"h2_{t}", tag="h2", bufs=2)
                nc.vector.tensor_tensor(h2[:, :], ga[:, 2 * BHC:3 * BHC], th1[:, :], MUL)
                if t == SEQ - 1:
                    h2_last = h2

                h1p, h2p, c0p, c1p = h1, h2, c0, c1

            # ---- final linear head ----
            Pf = ps.tile([2, BHC], F32, name="Pf", tag="P0i")
            nc.tensor.matmul(Pf[:, :], tfcw[:, :], h2_last[:, :], start=True, stop=True)
            ob = sb.tile([2, BHC], F32, name="ob")
            nc.scalar.activation(ob[:, :], Pf[:, :], AF.Identity, bias=tfcb[:, 0:1])
            nc.sync.dma_start(out[:, :], ob[:, :])

    nc.compile()
    return nc


def _prep_weights(w_ih0, w_hh0, b_ih0, b_hh0, w_ih1, w_hh1, b_ih1, b_hh1, fc_w, fc_b):
    """Host-side packing.  Device gate order: L0 [i, g, f, o] (k: 0=i,1=g,2=f,
    3=o), L1 [i, g, o, f].  PyTorch gate order is (i, f, g, o)."""
    H = HIDDEN
    GATES0 = [0, 2, 1, 3]       # device k -> pytorch gate for L0 [i, g, f, o]
    GATES1 = [0, 2, 3, 1]       # device k -> pytorch gate for L1 [i, g, o, f]
    b0 = (b_ih0 + b_hh0).reshape(4, H)
    b1 = (b_ih1 + b_hh1).reshape(4, H)
    wi0 = w_ih0.reshape(4, H, INPUT)
    wh0_ = w_hh0.reshape(4, H, H)
    wi1 = w_ih1.reshape(4, H, H)
    wh1_ = w_hh1.reshape(4, H, H)

    wx0 = np.zeros((4, 2 * D1, 128), np.float32)
    wh0 = np.zeros((4, 128, 128), np.float32)
    w1a = np.zeros((4, 128, 128), np.float32)
    w1b = np.zeros((4, 128, 128), np.float32)
    b1st = np.zeros((4, 1, 128), np.float32)
    for k, gi in enumerate(GATES0):
        sc = 2.0 if gi == 2 else 1.0      # pytorch gate 2 = g: pre-scale x2
        wt = sc * wi0[gi].T               # [INPUT, H]
        wx0[k, :INPUT, 0:H] = wt
        wx0[k, INPUT, 0:H] = sc * b0[gi]
        wx0[k, D1:D1 + INPUT, H:2 * H] = wt
        wx0[k, D1 + INPUT, H:2 * H] = sc * b0[gi]
        wh0[k, 0:H, 0:H] = sc * wh0_[gi].T
        wh0[k, H:2 * H, H:2 * H] = sc * wh0_[gi].T
    for k, gi in enumerate(GATES1):
        sc = 2.0 if gi == 2 else 1.0      # pytorch gate 2 = g: pre-scale x2
        w1a[k, 0:H, 0:H] = sc * wi1[gi].T
        w1a[k, H:2 * H, H:2 * H] = sc * wi1[gi].T
        w1b[k, 0:H, 0:H] = sc * wh1_[gi].T
        w1b[k, H:2 * H, H:2 * H] = sc * wh1_[gi].T
        b1st[k, 0, 0:H] = sc * b1[gi]
        b1st[k, 0, H:2 * H] = sc * b1[gi]

    fcw = np.zeros((128, 2), np.float32)
    fcw[0:H, 0] = fc_w[0]
    fcw[H:2 * H, 1] = fc_w[0]
    fcb = np.full((2, 1), np.float32(fc_b[0]), np.float32)
    return wx0, wh0, w1a, w1b, b1st, fcw, fcb


def run_full(x, w_ih0, w_hh0, b_ih0, b_hh0, w_ih1, w_hh1, b_ih1, b_hh1, fc_w, fc_b,
             trace=False):
    """Run the full problem on 8 cores; returns (output [BATCH], BassKernelResults)."""
    from concourse.bass_utils import run_bass_kernel_spmd

    x = np.asarray(x, np.float32)
    args = [np.asarray(a, np.float32) for a in
            (w_ih0, w_hh0, b_ih0, b_hh0, w_ih1, w_hh1, b_ih1, b_hh1, fc_w, fc_b)]
    wx0, wh0, w1a, w1b, b1st, fcw, fcb = _prep_weights(*args)

    if "nc" not in _CACHE:
        _CACHE["nc"] = _build_module()
    nc = _CACHE["nc"]

    in_maps = []
    for c in range(NCORES):
        xs = x[c * BCORE:(c + 1) * BCORE]                  # [BCORE, SEQ, INPUT]
        xT = np.empty((SEQ, 2 * D1, BHC), np.float32)
        xT[:, :INPUT, :] = xs[0:BHC].transpose(1, 2, 0)
        xT[:, INPUT, :] = 1.0
        xT[:, D1:D1 + INPUT, :] = xs[BHC:2 * BHC].transpose(1, 2, 0)
        xT[:, D1 + INPUT, :] = 1.0
        in_maps.append({
            "xT": xT, "wx0": wx0, "wh0": wh0, "w1a": w1a, "w1b": w1b,
            "b1st": b1st, "ones": np.ones((1, BHC), np.float32),
            "fcw": fcw, "fcb": fcb,
        })

    res = run_bass_kernel_spmd(nc, in_maps, core_ids=list(range(NCORES)), trace=trace)
    outs = []
    for r in res.results:
        o = r["out"]                        # [2, BHC]: (half, col)
        outs.append(o.reshape(BCORE))
    return np.concatenate(outs, axis=0).astype(np.float32), res


def kernel(x, w_ih0, w_hh0, b_ih0, b_hh0, w_ih1, w_hh1, b_ih1, b_hh1, fc_w, fc_b):
    out, _ = run_full(x, w_ih0, w_hh0, b_ih0, b_hh0,
                      w_ih1, w_hh1, b_ih1, b_hh1, fc_w, fc_b)
    return out


# revision 17
# speedup vs baseline: 1.3742x; 1.3742x over previous
"""Trainium2 Bass kernel for a 2-layer LSTM (batch 8192, seq 128, in 32, hidden 64)
with a final linear head producing one logit per batch element.

Strategy: pure data parallel over 8 NeuronCores (1024 batch each), weights
replicated.  The input projection is folded into the recurrent step (no
[B,T,4H] materialization) so HBM traffic is ~one read of x.

The ACT (scalar) engine is the bottleneck: the LSTM needs 5 transcendental
values per hidden unit per layer-step (4 gate sigmoids + tanh(c); tanh(g) is
computed as 2*sigmoid(2g)-1 with gate weights pre-scaled by 2), and ACT
streams 1 column/cycle regardless of dtype.  So ACT instructions are made as
wide as the recurrence allows, and everything else is kept off its critical
path:

- 2 pipeline chains of 512 batch (x2 partition halves of 256 columns),
  half-phase offset: chain B executes its layer-1 stage while chain A
  executes layer 0, so each chain's recurrence latency hides under the other
  chain's ACT work.
- Per chain-timestep ACT runs only 3 instructions: sigmoid over all 4 L0
  gates [128,1024], one merged tanh over [c1(t-1) | c0(t)] [128,512], and
  sigmoid over all 4 L1 gates [128,1024].
- L1's bias (forget gate +1) is accumulated into PSUM by one rank-1 ones
  matmul so the L1 sigmoid needs no per-block bias split.
- The f*c_prev product of layer 1 runs on the GpSimd (Pool) engine to keep
  DVE under the ACT shadow.  Matmuls use float32r (full-rate fp32).
"""

import numpy as np

INPUT = 32
HIDDEN = 64
BATCH = 8192
SEQ = 128
NCORES = 8
BCORE = BATCH // NCORES      # 1024
NCH = 2                      # pipeline chains per core
BHC = BCORE // (2 * NCH)     # 256 columns per chain (x2 partition halves)
D1 = INPUT + 1               # x rows + ones row

_CACHE = {}


def _build_module(b1_const):
    """b1_const: length-4 list of per-gate constant L1 biases (device L1 gate
    order [i, g, o, f]); None entries mean a non-constant bias vector (falls
    back to per-gate rank-1 matmuls from b1st)."""
    import concourse.bacc as bacc
    import concourse.mybir as mybir
    import concourse.tile as tile

    F32 = mybir.dt.float32
    F32R = mybir.dt.float32r
    AF = mybir.ActivationFunctionType
    MUL = mybir.AluOpType.mult
    ADD = mybir.AluOpType.add
    SUB = mybir.AluOpType.subtract

    nc = bacc.Bacc()
    # L0 gate blocks in P0: [i, g, f, o];  L1 gate blocks in P1: [i, g, o, f].
    xT = nc.dram_tensor("xT", [SEQ, NCH, 2 * D1, BHC], F32R, kind="ExternalInput")
    wx0 = nc.dram_tensor("wx0", [4, 2 * D1, 128], F32R, kind="ExternalInput")
    wh0 = nc.dram_tensor("wh0", [4, 128, 128], F32R, kind="ExternalInput")
    w1a = nc.dram_tensor("w1a", [4, 128, 128], F32R, kind="ExternalInput")
    w1b = nc.dram_tensor("w1b", [4, 128, 128], F32R, kind="ExternalInput")
    b1st = nc.dram_tensor("b1st", [4, 1, 128], F32R, kind="ExternalInput")
    bpat = nc.dram_tensor("bpat", [1, 4 * BHC], F32R, kind="ExternalInput")
    ones = nc.dram_tensor("ones", [1, 4 * BHC], F32R, kind="ExternalInput")
    fcw = nc.dram_tensor("fcw", [128, 2], F32R, kind="ExternalInput")
    fcb = nc.dram_tensor("fcb", [2, 1], F32, kind="ExternalInput")
    out = nc.dram_tensor("out", [NCH, 2, BHC], F32, kind="ExternalOutput")

    with tile.TileContext(nc) as tc:
        with (
            tc.tile_pool(name="wp", bufs=1) as wp,
            tc.tile_pool(name="sb", bufs=3) as sb,
            tc.tile_pool(name="ps", bufs=1, space="PSUM") as ps,
        ):
            twx = [wp.tile([2 * D1, 128], F32R, name=f"twx{g}", tag=f"twx{g}") for g in range(4)]
            twh = [wp.tile([128, 128], F32R, name=f"twh{g}", tag=f"twh{g}") for g in range(4)]
            t1a = [wp.tile([128, 128], F32R, name=f"t1a{g}", tag=f"t1a{g}") for g in range(4)]
            t1b = [wp.tile([128, 128], F32R, name=f"t1b{g}", tag=f"t1b{g}") for g in range(4)]
            tb1 = [wp.tile([1, 128], F32R, name=f"tb1{g}", tag=f"tb1{g}") for g in range(4)]
            tbp = wp.tile([1, 4 * BHC], F32R, name="tbp")
            tone = wp.tile([1, 4 * BHC], F32R, name="tone")
            tfcw = wp.tile([128, 2], F32R, name="tfcw")
            tfcb = wp.tile([2, 1], F32, name="tfcb")
            for g in range(4):
                nc.sync.dma_start(twx[g][:, :], wx0[g, :, :])
                nc.sync.dma_start(twh[g][:, :], wh0[g, :, :])
                nc.sync.dma_start(t1a[g][:, :], w1a[g, :, :])
                nc.sync.dma_start(t1b[g][:, :], w1b[g, :, :])
                nc.sync.dma_start(tb1[g][:, :], b1st[g, :, :])
            nc.sync.dma_start(tbp[:, :], bpat[0:1, :])
            nc.sync.dma_start(tone[:, :], ones[0:1, :])
            nc.sync.dma_start(tfcw[:, :], fcw[:, :])
            nc.sync.dma_start(tfcb[:, :], fcb[:, :])

            # Per-chain state (python handles to live tiles)
            st = [dict(h1=None, h2=None, ga=None, cc=None, ccn=None) for _ in range(NCH)]
            P0 = [None] * NCH

            def stage_L0(ch, t):
                s = st[ch]
                C = f"c{ch}_"
                xt = sb.tile([2 * D1, BHC], F32R, name=f"{C}xt{t}", tag=f"{C}xt", bufs=4)
                nc.sync.dma_start(xt[:, :], xT[t, ch, :, :])

                P = ps.tile([128, 4 * BHC], F32, name=f"{C}P0_{t}", tag=f"{C}P0", bufs=1)
                for g in range(4):
                    blk = slice(g * BHC, (g + 1) * BHC)
                    nc.tensor.matmul(P[:, blk], twx[g][:, :], xt[:, :],
                                     start=True, stop=(t == 0))
                    if t > 0:
                        nc.tensor.matmul(P[:, blk], twh[g][:, :], s["h1"][:, :],
                                         start=False, stop=True)

                # sigma over [i, g, f, o]
                sg = sb.tile([128, 4 * BHC], F32, name=f"{C}sg{t}", tag=f"{C}sg", bufs=2)
                nc.scalar.activation(sg[:, :], P[:, :], AF.Sigmoid)

                # ig = (sigma(2g) - 0.5) * sigma(i) = i*tanh(g)/2
                ig0 = sb.tile([128, BHC], F32, name=f"{C}ig0_{t}", tag=f"{C}ig0", bufs=2)
                nc.vector.scalar_tensor_tensor(
                    ig0[:, :], sg[:, BHC:2 * BHC], 0.5, sg[:, 0:BHC], SUB, MUL)
                # cc tile layout: [c1(t-1) | c0(t)]
                cc = s["cc"]
                if t == 0:
                    nc.vector.tensor_scalar_mul(cc[:, BHC:2 * BHC], ig0[:, :], 2.0)
                else:
                    ccp = s["ccp"]
                    fc0 = sb.tile([128, BHC], F32, name=f"{C}fc0_{t}", tag=f"{C}fc0", bufs=2)
                    nc.vector.tensor_tensor(fc0[:, :], sg[:, 2 * BHC:3 * BHC],
                                            ccp[:, BHC:2 * BHC], MUL)
                    nc.vector.scalar_tensor_tensor(
                        cc[:, BHC:2 * BHC], ig0[:, :], 2.0, fc0[:, :], MUL, ADD)

                # merged tanh over [c1(t-1) | c0(t)]  (t=0: right half only)
                th = sb.tile([128, 2 * BHC], F32, name=f"{C}th{t}", tag=f"{C}th", bufs=2)
                if t == 0:
                    nc.scalar.activation(th[:, BHC:2 * BHC], cc[:, BHC:2 * BHC], AF.Tanh)
                else:
                    nc.scalar.activation(th[:, :], cc[:, :], AF.Tanh)

                h1 = sb.tile([128, BHC], F32R, name=f"{C}h1_{t}", tag=f"{C}h1", bufs=2)
                nc.vector.tensor_tensor(h1[:, :], sg[:, 3 * BHC:4 * BHC],
                                        th[:, BHC:2 * BHC], MUL)
                s["h1"] = h1
                if t > 0:
                    # h2(t-1) = sigma_o1(t-1) * tanh(c1(t-1))
                    h2 = sb.tile([128, BHC], F32R, name=f"{C}h2_{t-1}", tag=f"{C}h2", bufs=2)
                    nc.vector.tensor_tensor(h2[:, :], s["ga"][:, 2 * BHC:3 * BHC],
                                            th[:, 0:BHC], MUL)
                    s["h2"] = h2

            def stage_L1(ch, t):
                s = st[ch]
                C = f"c{ch}_"
                P = ps.tile([128, 4 * BHC], F32, name=f"{C}P1_{t}", tag=f"{C}P1", bufs=1)
                for g in range(4):
                    blk = slice(g * BHC, (g + 1) * BHC)
                    nc.tensor.matmul(P[:, blk], t1a[g][:, :], s["h1"][:, :],
                                     start=True, stop=False)
                    if t > 0:
                        nc.tensor.matmul(P[:, blk], t1b[g][:, :], s["h2"][:, :],
                                         start=False, stop=False)
                    # per-gate bias as a rank-1 ones matmul accumulation
                    nc.tensor.matmul(P[:, blk], tb1[g][:, :], tone[:, 0:BHC],
                                     start=False, stop=True)

                ga = sb.tile([128, 4 * BHC], F32, name=f"{C}ga{t}", tag=f"{C}ga", bufs=2)
                nc.scalar.activation(ga[:, :], P[:, :], AF.Sigmoid)
                s["ga"] = ga

                ig1 = sb.tile([128, BHC], F32, name=f"{C}ig1_{t}", tag=f"{C}ig1", bufs=2)
                nc.vector.scalar_tensor_tensor(
                    ig1[:, :], ga[:, BHC:2 * BHC], 0.5, ga[:, 0:BHC], SUB, MUL)
                # c1(t) goes into the NEXT timestep's cc tile (left half)
                ccn = sb.tile([128, 2 * BHC], F32, name=f"{C}cc{t+1}", tag=f"{C}cc", bufs=3)
                if t == 0:
                    nc.vector.tensor_scalar_mul(ccn[:, 0:BHC], ig1[:, :], 2.0)
                else:
                    fc1 = sb.tile([128, BHC], F32, name=f"{C}fc1_{t}", tag=f"{C}fc1", bufs=2)
                    nc.vector.tensor_tensor(fc1[:, :], ga[:, 3 * BHC:4 * BHC],
                                            s["cc"][:, 0:BHC], MUL)
                    nc.vector.scalar_tensor_tensor(
                        ccn[:, 0:BHC], ig1[:, :], 2.0, fc1[:, :], MUL, ADD)
                s["ccp"] = s["cc"]
                s["cc"] = ccn

            # prime each chain's first cc tile
            for ch in range(NCH):
                st[ch]["cc"] = sb.tile([128, 2 * BHC], F32, name=f"c{ch}_cc0",
                                       tag=f"c{ch}_cc", bufs=3)

            # Half-phase offset schedule: A.L0(t), B.L1(t-1), B.L0(t), A.L1(t)
            for t in range(SEQ):
                stage_L0(0, t)
                if t > 0:
                    stage_L1(1, t - 1)
                stage_L0(1, t)
                stage_L1(0, t)
            stage_L1(1, SEQ - 1)

            # ---- final h2(SEQ-1) + linear head per chain ----
            for ch in range(NCH):
                s = st[ch]
                C = f"c{ch}_"
                thf = sb.tile([128, BHC], F32, name=f"{C}thf", tag=f"{C}th", bufs=2)
                nc.scalar.activation(thf[:, :], s["cc"][:, 0:BHC], AF.Tanh)
                h2 = sb.tile([128, BHC], F32R, name=f"{C}h2f", tag=f"{C}h2", bufs=2)
                nc.vector.tensor_tensor(h2[:, :], s["ga"][:, 2 * BHC:3 * BHC],
                                        thf[:, :], MUL)
                Pf = ps.tile([2, BHC], F32, name=f"Pf{ch}", tag=f"{C}P0")
                nc.tensor.matmul(Pf[:, :], tfcw[:, :], h2[:, :], start=True, stop=True)
                ob = sb.tile([2, BHC], F32, name=f"ob{ch}")
                nc.scalar.activation(ob[:, :], Pf[:, :], AF.Identity, bias=tfcb[:, 0:1])
                nc.sync.dma_start(out[ch, :, :], ob[:, :])

    nc.compile()
    return nc


def _prep_weights(w_ih0, w_hh0, b_ih0, b_hh0, w_ih1, w_hh1, b_ih1, b_hh1, fc_w, fc_b):
    """Host-side packing.  Device gate order: L0 [i, g, f, o], L1 [i, g, o, f]
    (PyTorch order is i, f, g, o)."""
    H = HIDDEN
    GATES0 = [0, 2, 1, 3]       # device k -> pytorch gate for L0 [i, g, f, o]
    GATES1 = [0, 2, 3, 1]       # device k -> pytorch gate for L1 [i, g, o, f]
    b0 = (b_ih0 + b_hh0).reshape(4, H)
    b1 = (b_ih1 + b_hh1).reshape(4, H)
    wi0 = w_ih0.reshape(4, H, INPUT)
    wh0_ = w_hh0.reshape(4, H, H)
    wi1 = w_ih1.reshape(4, H, H)
    wh1_ = w_hh1.reshape(4, H, H)

    wx0 = np.zeros((4, 2 * D1, 128), np.float32)
    wh0 = np.zeros((4, 128, 128), np.float32)
    w1a = np.zeros((4, 128, 128), np.float32)
    w1b = np.zeros((4, 128, 128), np.float32)
    b1st = np.zeros((4, 1, 128), np.float32)
    b1c = [None] * 4
    for k, gi in enumerate(GATES0):
        sc = 2.0 if gi == 2 else 1.0      # pytorch gate 2 = g: pre-scale x2
        wt = sc * wi0[gi].T               # [INPUT, H]
        wx0[k, :INPUT, 0:H] = wt
        wx0[k, INPUT, 0:H] = sc * b0[gi]
        wx0[k, D1:D1 + INPUT, H:2 * H] = wt
        wx0[k, D1 + INPUT, H:2 * H] = sc * b0[gi]
        wh0[k, 0:H, 0:H] = sc * wh0_[gi].T
        wh0[k, H:2 * H, H:2 * H] = sc * wh0_[gi].T
    for k, gi in enumerate(GATES1):
        sc = 2.0 if gi == 2 else 1.0
        w1a[k, 0:H, 0:H] = sc * wi1[gi].T
        w1a[k, H:2 * H, H:2 * H] = sc * wi1[gi].T
        w1b[k, 0:H, 0:H] = sc * wh1_[gi].T
        w1b[k, H:2 * H, H:2 * H] = sc * wh1_[gi].T
        b1st[k, 0, 0:H] = sc * b1[gi]
        b1st[k, 0, H:2 * H] = sc * b1[gi]
        if np.all(b1[gi] == b1[gi][0]):
            b1c[k] = float(sc * b1[gi][0])

    fcw = np.zeros((128, 2), np.float32)
    fcw[0:H, 0] = fc_w[0]
    fcw[H:2 * H, 1] = fc_w[0]
    fcb = np.full((2, 1), np.float32(fc_b[0]), np.float32)
    return wx0, wh0, w1a, w1b, b1st, b1c, fcw, fcb


def run_full(x, w_ih0, w_hh0, b_ih0, b_hh0, w_ih1, w_hh1, b_ih1, b_hh1, fc_w, fc_b,
             trace=False):
    """Run the full problem on 8 cores; returns (output [BATCH], BassKernelResults)."""
    from concourse.bass_utils import run_bass_kernel_spmd

    x = np.asarray(x, np.float32)
    args = [np.asarray(a, np.float32) for a in
            (w_ih0, w_hh0, b_ih0, b_hh0, w_ih1, w_hh1, b_ih1, b_hh1, fc_w, fc_b)]
    wx0, wh0, w1a, w1b, b1st, b1c, fcw, fcb = _prep_weights(*args)

    b1_const = b1c if all(v is not None for v in b1c) else None
    key = ("const", tuple(b1c)) if b1_const is not None else ("vec",)
    if key not in _CACHE:
        _CACHE.clear()
        _CACHE[key] = _build_module(b1_const)
    nc = _CACHE[key]

    bpat = np.zeros((1, 4 * BHC), np.float32)
    if b1_const is not None:
        for g in range(4):
            bpat[0, g * BHC:(g + 1) * BHC] = b1_const[g]

    in_maps = []
    for c in range(NCORES):
        xs = x[c * BCORE:(c + 1) * BCORE]                  # [BCORE, SEQ, INPUT]
        xT = np.empty((SEQ, NCH, 2 * D1, BHC), np.float32)
        for ch in range(NCH):
            a0 = ch * BHC
            b0_ = BCORE // 2 + ch * BHC
            xT[:, ch, :INPUT, :] = xs[a0:a0 + BHC].transpose(1, 2, 0)
            xT[:, ch, INPUT, :] = 1.0
            xT[:, ch, D1:D1 + INPUT, :] = xs[b0_:b0_ + BHC].transpose(1, 2, 0)
            xT[:, ch, D1 + INPUT, :] = 1.0
        in_maps.append({
            "xT": xT, "wx0": wx0, "wh0": wh0, "w1a": w1a, "w1b": w1b,
            "b1st": b1st, "bpat": bpat, "ones": np.ones((1, 4 * BHC), np.float32),
            "fcw": fcw, "fcb": fcb,
        })

    res = run_bass_kernel_spmd(nc, in_maps, core_ids=list(range(NCORES)), trace=trace)
    outs = []
    for r in res.results:
        o = r["out"]                        # [NCH, 2, BHC]: (chain, half, col)
        # per-core batch order: [ch0 halfA, ch1 halfA, ch0 halfB, ch1 halfB]
        outs.append(o.transpose(1, 0, 2).reshape(BCORE))
    return np.concatenate(outs, axis=0).astype(np.float32), res


def kernel(x, w_ih0, w_hh0, b_ih0, b_hh0, w_ih1, w_hh1, b_ih1, b_hh1, fc_w, fc_b):
    out, _ = run_full(x, w_ih0, w_hh0, b_ih0, b_hh0,
                      w_ih1, w_hh1, b_ih1, b_hh1, fc_w, fc_b)
    return out
